# revision 1
# baseline (speedup 1.0000x reference)
"""Trainium2 Bass kernel for nn_InteractionModule (GNN message passing).

v2 strategy (8 NeuronCores, SPMD, no collectives):
 - Nodes sharded 8 x 6250 by dst; edges assigned to the core owning dst,
   grouped by (src-quarter, dst-chunk), padded to 128-edge subtiles with
   equal counts across cores (one NEFF serves all).
 - Phase 1 (replicated): each core computes the full ssp table
   yE[n] = ssp(ssp(x) @ W_diff.T + b_diff) for all N nodes in ROW layout
   (matmul-as-transpose: per 125-node block, stationary = xa columns,
   moving = W_diffT; bias injected via a K=1 ones x brow matmul into
   PSUM).  Table stored fp16 into 4 quarter DRAM tensors so phase 2 of
   quarter q only depends on phase-1 windows of quarter q (overlap).
 - Phase 2: per 16-subtile group one batched indirect DMA gathers 2048
   ssp rows; gate = eaT @ G_w on PE; msg = ssp * gate (DVE); one-hot
   scatter via fp16 iota-compare (Pool/DVE) + PE matmul accumulating
   aggrT[f, node] in PSUM per (quarter, chunk) group; group results are
   added into z_sT (which phase 1b pre-filled with ssp(z_same)).
 - Phase 3: residual stack + out head in [f, node] layout (fp16 moving
   matmuls, exp/ln composite ssp), PE-transposed back to rows.
 - Single activation-table load (exp+ln combined set) pre-placed to
   avoid per-switch table reloads.
"""

import numpy as np

N, E, F, K, R = 50000, 600000, 128, 64, 3
NC_ = 8
NSH = N // NC_            # 6250 nodes per core
CHUNK = 256               # scatter window (one-hot width)
NCHUNK = (NSH + CHUNK - 1) // CHUNK   # 25
NQ = 4                    # src quarters (for phase1/phase2 overlap)
WIN = 2000                # phase-1 window (25 windows), 16 blocks of 125
BLK = 125
QWIN = [6, 6, 6, 7]       # phase-1 windows per quarter
QB = [0, 12000, 24000, 36000, 50000]  # quarter row bounds (src space)
WIN3 = 1024               # phase-1b/3 window (col layout)
GG = 16                   # subtiles per gather group
LOG2 = float(np.log(2.0))

_cache = {}


def _prep(edge_index, edge_attr):
    """Host-side sharding: per-core edge arrays + structure lists."""
    src = np.asarray(edge_index[0], dtype=np.int64)
    dst = np.asarray(edge_index[1], dtype=np.int64)
    core = dst // NSH
    dstl = dst - core * NSH
    ea = np.asarray(edge_attr, dtype=np.float32)

    chunk = dstl // CHUNK
    q = np.searchsorted(np.asarray(QB[1:NQ]), src, side="right")
    key = (core * NQ + q) * NCHUNK + chunk
    order = np.argsort(key, kind="stable")
    counts = np.bincount(key[order], minlength=NC_ * NQ * NCHUNK)
    counts = counts.reshape(NC_, NQ, NCHUNK)
    st = (counts + 127) // 128
    st_max = st.max(axis=0)                  # [NQ, NCHUNK] equalized

    # per-quarter subtile streams, padded to multiple of 4 (supertiles)
    q_sizes = st_max.sum(axis=1)
    q_pad = (-q_sizes) % 4
    S = int((q_sizes + q_pad).sum())
    G = S // 4
    Gp = (G + 1) // 2

    chunk_of = np.zeros(S, np.int64)
    firsts = np.zeros(S, bool)
    lasts = np.zeros(S, bool)
    q_start = np.zeros(NQ + 1, np.int64)
    pos = 0
    for qq in range(NQ):
        q_start[qq] = pos
        last_c = -1
        for c in range(NCHUNK):
            n_t = int(st_max[qq, c])
            if n_t == 0:
                continue
            chunk_of[pos:pos + n_t] = c
            firsts[pos] = True
            lasts[pos + n_t - 1] = True
            pos += n_t
            last_c = c
        if q_pad[qq]:
            # extend the last chunk's accumulation group with pad subtiles
            assert last_c >= 0
            lasts[pos - 1] = False
            chunk_of[pos:pos + q_pad[qq]] = last_c
            pos += int(q_pad[qq])
            lasts[pos - 1] = True
    q_start[NQ] = pos
    assert pos == S

    # subtile slot offset for each (q, chunk)
    slot = np.zeros((NQ, NCHUNK), np.int64)
    for qq in range(NQ):
        p2 = q_start[qq]
        for c in range(NCHUNK):
            slot[qq, c] = p2
            p2 += int(st_max[qq, c])

    # flat per-core padded arrays in (subtile, lane) order
    src_a = np.zeros((NC_, S * 128), np.int32)
    dstf_a = np.full((NC_, S * 128), -1.0, np.float32)
    ea_a = np.zeros((NC_, S * 128, K), np.float32)
    cum = np.zeros(NC_ * NQ * NCHUNK + 1, np.int64)
    np.cumsum(counts.ravel(), out=cum[1:])
    for c in range(NC_):
        for qq in range(NQ):
            for ch in range(NCHUNK):
                k0 = cum[(c * NQ + qq) * NCHUNK + ch]
                n_e = counts[c, qq, ch]
                if n_e == 0:
                    continue
                sl = order[k0:k0 + n_e]
                p0 = slot[qq, ch] * 128
                src_a[c, p0:p0 + n_e] = (src[sl] - QB[qq]).astype(np.int32)
                dstf_a[c, p0:p0 + n_e] = (dstl[sl] - ch * CHUNK).astype(np.float32)
                ea_a[c, p0:p0 + n_e] = ea[sl]

    # device layouts
    # dma_gather idx layout: idx i at [16*r + i % 16, i // 16] (8 replicas)
    s16 = src_a.astype(np.int16).reshape(NC_, S * 8, 16).transpose(0, 2, 1)
    src_d = np.tile(s16, (1, 8, 1)).copy()          # [NC, 128, S*8] int16
    dstf_d = dstf_a.reshape(NC_, S, 128).transpose(0, 2, 1).copy()
    # eaT packed: supertile pairs in upper/lower 64 partitions
    eaT = ea_a.reshape(NC_, G, 512, K).transpose(0, 1, 3, 2)  # [NC, G, K, 512]
    ea_d = np.zeros((NC_, 128, Gp * 512), np.float16)
    ev = eaT[:, 0::2]
    ea_d[:, :K, :ev.shape[1] * 512] = ev.transpose(0, 2, 1, 3).reshape(NC_, K, -1)
    od = eaT[:, 1::2]
    ea_d[:, K:2 * K, :od.shape[1] * 512] = od.transpose(0, 2, 1, 3).reshape(NC_, K, -1)

    # gather groups: (quarter, start subtile, count)
    ggroups = []
    for qq in range(NQ):
        s0, s1 = int(q_start[qq]), int(q_start[qq + 1])
        p2 = s0
        while p2 < s1:
            cnt = min(GG, s1 - p2)
            ggroups.append((qq, p2, cnt))
            p2 += cnt

    meta = dict(S=S, G=G, Gp=Gp,
                chunk_of=chunk_of.tolist(),
                firsts=firsts.tolist(), lasts=lasts.tolist(),
                q_start=q_start.tolist(),
                ggroups=ggroups)
    return src_d, dstf_d, ea_d, meta


def _build(nc, meta):
    import contextlib
    import concourse.bass as bass
    import concourse.mybir as mybir
    import concourse.tile as tile
    from concourse.masks import make_identity
    from concourse.hw_specs import get_activation_tables

    F32, F16, I32 = mybir.dt.float32, mybir.dt.float16, mybir.dt.int32
    AF, ALU = mybir.ActivationFunctionType, mybir.AluOpType
    S, G, Gp = meta["S"], meta["G"], meta["Gp"]
    chunk_of, firsts, lasts = meta["chunk_of"], meta["firsts"], meta["lasts"]
    ggroups = meta["ggroups"]

    xT16 = nc.dram_tensor("xT16", [F, N], F16, kind="ExternalInput").ap()
    xTo = nc.dram_tensor("xTo", [F, NSH], F32, kind="ExternalInput").ap()
    wpack16 = nc.dram_tensor("wpack16", [F, 9 * F], F16, kind="ExternalInput").ap()
    bpack = nc.dram_tensor("bpack", [F, 16], F32, kind="ExternalInput").ap()
    bdiv16_in = nc.dram_tensor("bdiv16", [F, F], F16, kind="ExternalInput").ap()
    gw2 = nc.dram_tensor("gw2", [128, 128], F16, kind="ExternalInput").ap()
    iota_in = nc.dram_tensor("iota_in", [128, CHUNK], F16, kind="ExternalInput").ap()
    src_in = nc.dram_tensor("src_in", [128, S * 8], mybir.dt.int16,
                            kind="ExternalInput").ap()
    dstf_in = nc.dram_tensor("dstf_in", [128, S], F32, kind="ExternalInput").ap()
    ea_in = nc.dram_tensor("ea_in", [128, Gp * 512], F16, kind="ExternalInput").ap()
    out0 = nc.dram_tensor("out0", [NSH, F], F32, kind="ExternalOutput").ap()
    out1 = nc.dram_tensor("out1", [NSH, F], F32, kind="ExternalOutput").ap()

    # combined exp+ln activation table set, loaded once
    tables = list(get_activation_tables(nc.m.arch).items())
    set_id = next(i for i, (_, fns) in enumerate(tables)
                  if AF.Exp in fns and AF.Ln in fns)

    with tile.TileContext(nc) as tc, contextlib.ExitStack() as ctx:
        const = ctx.enter_context(tc.tile_pool(name="const", bufs=1))
        big = ctx.enter_context(tc.tile_pool(name="big", bufs=1))
        wk1 = ctx.enter_context(tc.tile_pool(name="wk1", bufs=2))
        wk2 = ctx.enter_context(tc.tile_pool(name="wk2", bufs=3))
        wk3 = ctx.enter_context(tc.tile_pool(name="wk3", bufs=2))
        ps_zr = ctx.enter_context(tc.tile_pool(name="pszr", bufs=3, space="PSUM"))
        ps_g = ctx.enter_context(tc.tile_pool(name="psg", bufs=3, space="PSUM"))
        ps_ag = ctx.enter_context(tc.tile_pool(name="psag", bufs=2, space="PSUM"))
        dram = ctx.enter_context(tc.tile_pool(name="dram", bufs=1, space="DRAM"))

        nc.scalar.add_instruction(mybir.InstLoadActFuncSet(
            name=nc.get_next_instruction_name(), ins=[], outs=[],
            act_func_set_id=set_id))

        yEq = [dram.tile([QB[i + 1] - QB[i], F], F16, name=f"yEq{i}")
               for i in range(NQ)]

        # first phase-1 x tiles load ahead of the weight consts so the
        # Activation engine starts immediately
        xt_tiles = {}

        def load_xt(wi):
            t = wk1.tile([128, WIN], F16, tag="xt")
            nc.sync.dma_start(t[:], xT16[:, wi * WIN:(wi + 1) * WIN])
            xt_tiles[wi] = t

        load_xt(0)

        # ---- consts ----
        wp16 = const.tile([F, 9 * F], F16)
        nc.sync.dma_start(wp16[:], wpack16)
        bp = const.tile([F, 16], F32)
        nc.sync.dma_start(bp[:], bpack)
        bdiv = const.tile([F, F], F16)
        nc.sync.dma_start(bdiv[:], bdiv16_in)
        gw = const.tile([128, 128], F16)
        iota = const.tile([128, CHUNK], F16)
        srcs16 = const.tile([128, S * 8], mybir.dt.int16)
        dstf = const.tile([128, S], F32)
        ident = const.tile([128, 128], F32)
        make_identity(nc, ident[:])
        half = const.tile([128, 1], F32)
        nc.gpsimd.memset(half[:], 0.5)
        ones16 = const.tile([128, 128], F16)
        nc.gpsimd.memset(ones16[:], 1.0)

        W_diffT = wp16[:, 0:F]
        W_sameT = wp16[:, F:2 * F]
        W1T = [wp16[:, (2 + i) * F:(3 + i) * F] for i in range(3)]
        W2T = [wp16[:, (5 + i) * F:(6 + i) * F] for i in range(3)]
        W_lastT = wp16[:, 8 * F:9 * F]
        b_same = bp[:, 1:2]
        b1 = [bp[:, 2 + i:3 + i] for i in range(3)]
        b2 = [bp[:, 5 + i:6 + i] for i in range(3)]
        b_last = bp[:, 8:9]
        uT = bp[:, 9:10]

        z_sT = big.tile([128, NSH], F32)
        xuT = big.tile([128, NSH], F16)
        aggrT = big.tile([128, NSH], F16)
        nc.gpsimd.memset(aggrT[:], 0.0)

        def load_ph2_consts():
            nc.sync.dma_start(gw[:], gw2)
            nc.sync.dma_start(iota[:], iota_in)
            nc.sync.dma_start(srcs16[:], src_in)
            nc.sync.dma_start(dstf[:], dstf_in)

        # ---- phase 1b: own-shard ssp(z_same) into z_sT; xuT ----
        xo_tiles = {}

        def load_xo(n0):
            nw = min(WIN3, NSH - n0)
            t = wk1.tile([128, WIN3], F32, tag="xo")
            nc.sync.dma_start(t[:, :nw], xTo[:, n0:n0 + nw])
            xo_tiles[n0] = t

        def emit_phase1b():
          load_xo(0)
          for n0 in range(0, NSH, WIN3):
             nw = min(WIN3, NSH - n0)
             if n0 + WIN3 < NSH:
                 load_xo(n0 + WIN3)
             xo = xo_tiles.pop(n0)
             nc.gpsimd.tensor_scalar_mul(xuT[:, n0:n0 + nw], xo[:, :nw], uT)
             ex = wk1.tile([128, WIN], F32, tag="ex", bufs=1)
             nc.scalar.activation(ex[:, :nw], xo[:, :nw], AF.Exp)
             xa = wk1.tile([128, WIN], F16, tag="xa")
             nc.scalar.activation(xa[:, :nw], ex[:, :nw], AF.Ln,
                                  bias=half[:, 0:1], scale=0.5)
             ez = wk1.tile([128, 2048], F32, tag="ez", bufs=1)
             for j in range(0, nw, 512):
                 bw = min(512, nw - j)
                 zps = ps_zr.tile([128, 512], F32, tag="zr")
                 nc.tensor.matmul(zps[:, :bw], W_sameT, xa[:, j:j + bw],
                                  start=True, stop=True, skip_group_check=True)
                 nc.scalar.activation(ez[:, j:j + bw], zps[:, :bw], AF.Exp,
                                      bias=b_same)
             nc.scalar.activation(z_sT[:, n0:n0 + nw], ez[:, :nw], AF.Ln,
                                  bias=half[:, 0:1], scale=0.5)

        # ---- phase 2 emission helper (one quarter's edge stream) ----
        EATQ = (max((((qe - 1) // 4) // 2) - ((qs // 4) // 2)
                    for (qs, qe) in zip(meta["q_start"][:-1], meta["q_start"][1:]))
                + 1)

        def phase2_groups(qq):
            aggr = None
            # one batched edge-attr load for the whole quarter
            pid_lo = (meta["q_start"][qq] // 4) // 2
            pid_hi = ((meta["q_start"][qq + 1] - 1) // 4) // 2
            etq = wk2.tile([128, EATQ * 512], F16, tag="eatq", bufs=1)
            nc.sync.dma_start(etq[:, 0:(pid_hi - pid_lo + 1) * 512],
                              ea_in[:, pid_lo * 512:(pid_hi + 1) * 512])
            for (gq, gs, gcnt) in ggroups:
                if gq != qq:
                    continue
                last_done = None
                yg = wk2.tile([128, GG, F], F16, tag="yg", bufs=4)
                nc.gpsimd.dma_gather(
                    out_ap=yg[:, 0:gcnt, :],
                    in_ap=yEq[qq][:],
                    idxs_ap=srcs16[:, gs * 8:(gs + gcnt) * 8],
                    num_idxs=gcnt * 128,
                    num_idxs_reg=gcnt * 128,
                    elem_size=128,
                    single_packet=False,
                )
                for h0 in range(0, gcnt, 4):
                  g = (gs + h0) // 4
                  pid = g // 2
                  et = etq[:, (pid - pid_lo) * 512:(pid - pid_lo + 1) * 512]
                  rb = 64 * (g % 2)
                  gps = ps_g.tile([128, 4, F], F32, tag="gate")
                  for t in range(4):
                    nc.tensor.matmul(gps[:, t, :],
                                     et[rb:rb + 64, 128 * t:128 * t + 128],
                                     gw[rb:rb + 64, :],
                                     start=True, stop=True,
                                     skip_group_check=True)
                  msg = wk2.tile([128, 4, F], F16, tag="msg", bufs=6)
                  nc.vector.tensor_tensor(
                    msg[:].rearrange("p a b -> p (a b)"),
                    yg[:, h0:h0 + 4, :].rearrange("p a b -> p (a b)"),
                    gps[:].rearrange("p a b -> p (a b)"),
                    ALU.mult)
                  for k in range(4):
                    s = gs + h0 + k
                    if firsts[s]:
                        aggr = ps_ag.tile([128, CHUNK], F32, tag="aggr")
                    m16 = wk2.tile([128, CHUNK], F16, tag="m16", bufs=8)
                    if qq == NQ - 1:
                        eng = nc.gpsimd if (s % 3) != 0 else nc.vector
                    else:
                        eng = nc.gpsimd if (s % 2) != 0 else nc.vector
                    eng.tensor_scalar(m16[:], iota[:],
                                      dstf[:, s:s + 1], 0.0,
                                      ALU.subtract, ALU.is_equal)
                    nc.tensor.matmul(aggr[:], msg[:, k, :], m16[:],
                                     start=bool(firsts[s]),
                                     stop=bool(lasts[s]),
                                     skip_group_check=True)
                    if lasts[s]:
                        c = chunk_of[s]
                        c0 = c * CHUNK
                        cw = min(CHUNK, NSH - c0)
                        nc.vector.tensor_tensor(
                            aggrT[:, c0:c0 + cw], aggrT[:, c0:c0 + cw],
                            aggr[:, :cw], ALU.add)
                        last_done = c
                yield last_done

        # ---- phase 1 (row layout, quarter DRAM tensors), with lagged
        # ---- phase-2 interleave for overlap ----
        w = 0
        p2gen = None
        for qq in range(NQ):
            for _ in range(QWIN[qq]):
                n0 = w * WIN
                n0q = n0 - QB[qq]
                if w + 1 < sum(QWIN):
                    load_xt(w + 1)
                xt = xt_tiles.pop(w)
                ex = wk1.tile([128, WIN], F32, tag="ex", bufs=1)
                nc.scalar.activation(ex[:], xt[:], AF.Exp)
                xa = wk1.tile([128, WIN], F16, tag="xa")
                nc.scalar.activation(xa[:], ex[:], AF.Ln,
                                     bias=half[:, 0:1], scale=0.5)
                ez = wk1.tile([128, 2048], F32, tag="ez", bufs=1)
                for sb in range(4):
                    zr = ps_zr.tile([128, 512], F32, tag="zr")
                    for t in range(4):
                        ns = sb * 500 + t * BLK
                        o = zr[0:BLK, 128 * t:128 * t + 128]
                        nc.tensor.matmul(o, ones16[:, 0:BLK], bdiv[:],
                                         start=True, stop=False,
                                         skip_group_check=True)
                        nc.tensor.matmul(o, xa[:, ns:ns + BLK], W_diffT,
                                         start=False, stop=True,
                                         skip_group_check=True)
                    nc.scalar.activation(ez[0:BLK, sb * 512:(sb + 1) * 512],
                                         zr[0:BLK, :], AF.Exp)
                y16 = wk1.tile([128, 2048], F16, tag="y16")
                nc.scalar.activation(y16[0:BLK, :], ez[0:BLK, :], AF.Ln,
                                     bias=half[0:BLK, 0:1], scale=0.5)
                nc.sync.dma_start(
                    yEq[qq][n0q:n0q + WIN].rearrange("(t p) f -> p t f", p=BLK),
                    y16[0:BLK, :].rearrange("p (t f) -> p t f", t=16))
                w += 1
                if p2gen is not None:
                    next(p2gen, None)
                    if qq == NQ - 1:
                        next(p2gen, None)
            if qq == 0:
                load_ph2_consts()
            if qq == NQ - 1:
                emit_phase1b()
            if p2gen is not None:
                for _ in p2gen:
                    pass
            p2gen = phase2_groups(qq)

        # ---- phase 3: residual stack + out head (col layout) ----
        def emit_rows(colT, col0, outdram, n0, nw):
            """Transpose [f, node] cols [col0, col0+nw) of colT into rows of
            outdram starting at row n0."""
            nb = (nw + 127) // 128
            orow = wk3.tile([128, 8, 128], F32, tag="orow")
            for j0 in range(0, nb, 4):
                jn = min(4, nb - j0)
                tp = ps_g.tile([128, 512], F32, tag="gate")
                for j in range(jn):
                    pw = min(128, nw - (j0 + j) * 128)
                    nc.tensor.transpose(tp[:pw, 128 * j:128 * j + 128],
                                        colT[:, col0 + (j0 + j) * 128:
                                             col0 + (j0 + j) * 128 + pw],
                                        ident[:])
                nc.vector.tensor_copy(
                    orow[:, j0:j0 + jn, :].rearrange("p a b -> p (a b)"),
                    tp[:, 0:jn * 128])
            if nw % 128 == 0:
                nc.sync.dma_start(
                    outdram[n0:n0 + nw].rearrange("(t p) f -> p t f", p=128),
                    orow[:, 0:nb, :])
            else:
                if nb > 1:
                    nc.sync.dma_start(
                        outdram[n0:n0 + (nb - 1) * 128].rearrange(
                            "(t p) f -> p t f", p=128),
                        orow[:, 0:nb - 1, :])
                pw = nw - (nb - 1) * 128
                nc.sync.dma_start(outdram[n0 + (nb - 1) * 128:n0 + nw],
                                  orow[0:pw, nb - 1, :])

        # layer-major emission across ALL windows: with 7 windows in
        # flight per layer, the Act->PE->DVE->Act chain latency of each
        # window hides under the other windows' work
        wins = [(n0, min(WIN3, NSH - n0)) for n0 in range(0, NSH, WIN3)]
        cur_of = {}

        def p3_layer(n0, nw, i):
            cur = cur_of[n0]
            e1 = wk3.tile([128, WIN3], F32, tag="e1", bufs=2)
            nc.scalar.activation(e1[:, :nw], cur, AF.Exp)
            s1 = wk3.tile([128, WIN3], F16, tag="s1", bufs=2)
            nc.scalar.activation(s1[:, :nw], e1[:, :nw], AF.Ln,
                                 bias=half[:, 0:1], scale=0.5)
            e2 = wk3.tile([128, WIN3], F32, tag="e1", bufs=2)
            for j in range(0, nw, 512):
                bw = min(512, nw - j)
                z1 = ps_zr.tile([128, 512], F32, tag="zr")
                nc.tensor.matmul(z1[:, :bw], W1T[i], s1[:, j:j + bw],
                                 start=True, stop=True, skip_group_check=True)
                nc.scalar.activation(e2[:, j:j + bw], z1[:, :bw], AF.Exp,
                                     bias=b1[i])
            s2 = wk3.tile([128, WIN3], F16, tag="s1", bufs=2)
            nc.scalar.activation(s2[:, :nw], e2[:, :nw], AF.Ln,
                                 bias=half[:, 0:1], scale=0.5)
            tw = wk3.tile([128, WIN3], F16, tag="tw", bufs=8)
            for j in range(0, nw, 512):
                bw = min(512, nw - j)
                z2 = ps_zr.tile([128, 512], F32, tag="zr")
                nc.tensor.matmul(z2[:, :bw], W2T[i], s2[:, j:j + bw],
                                 start=True, stop=True, skip_group_check=True)
                nc.vector.scalar_tensor_tensor(tw[:, j:j + bw], z2[:, :bw],
                                               b2[i], cur[:, j:j + bw],
                                               ALU.add, ALU.add)
            cur_of[n0] = tw[:, :nw]

        def p3_final(n0, nw):
            cur = cur_of[n0]
            ev = wk3.tile([128, WIN3], F32, tag="e1", bufs=2)
            nc.scalar.activation(ev[:, :nw], cur, AF.Exp)
            sv = wk3.tile([128, WIN3], F16, tag="s1", bufs=2)
            nc.scalar.activation(sv[:, :nw], ev[:, :nw], AF.Ln,
                                 bias=half[:, 0:1], scale=0.5)
            o0t = wk3.tile([128, WIN3], F32, tag="o0t", bufs=2)
            for j in range(0, nw, 512):
                bw = min(512, nw - j)
                zv = ps_zr.tile([128, 512], F32, tag="zr")
                nc.tensor.matmul(zv[:, :bw], W_lastT, sv[:, j:j + bw],
                                 start=True, stop=True, skip_group_check=True)
                nc.vector.scalar_tensor_tensor(o0t[:, j:j + bw], zv[:, :bw],
                                               b_last,
                                               xuT[:, n0 + j:n0 + j + bw],
                                               ALU.add, ALU.add)
            emit_rows(o0t, 0, out0, n0, nw)

        # drive the last quarter's stream, merging aggrT into z_sT for each
        # phase-3 window as soon as its chunks are fully aggregated
        def merge_win(idx):
            n0, nw = wins[idx]
            nc.vector.tensor_tensor(z_sT[:, n0:n0 + nw], z_sT[:, n0:n0 + nw],
                                    aggrT[:, n0:n0 + nw], ALU.add)
            cur_of[n0] = z_sT[:, n0:n0 + nw]

        def merge_need(idx):
            return (min((idx + 1) * WIN3, NSH) + CHUNK - 1) // CHUNK - 1

        done_chunk = -1
        next_merge = 0
        for c_done in p2gen:
            if c_done is not None:
                done_chunk = max(done_chunk, c_done)
            while next_merge < len(wins) and merge_need(next_merge) <= done_chunk:
                merge_win(next_merge)
                next_merge += 1
        while next_merge < len(wins):
            merge_win(next_merge)
            next_merge += 1
        for i in range(R):
            for n0, nw in wins:
                p3_layer(n0, nw, i)
            if i == 0:
                for n0, nw in wins:
                    emit_rows(z_sT, n0, out1, n0, nw)
        for n0, nw in wins:
            p3_final(n0, nw)
    return nc


def kernel(**inputs):
    import concourse.bacc as bacc
    from concourse import bass_utils

    x = np.asarray(inputs["x"], np.float32)
    src_d, dstf_d, ea_d, meta = _prep(inputs["edge_index"], inputs["edge_attr"])

    key = (meta["S"], meta["G"], tuple(meta["q_start"]),
           tuple(meta["chunk_of"]))
    if key not in _cache:
        nc = bacc.Bacc("TRN2", target_bir_lowering=False, debug=False,
                       enable_asserts=False, num_devices=NC_)
        _build(nc, meta)
        nc.compile()
        _cache[key] = nc
    nc = _cache[key]

    wpack16 = np.concatenate(
        [np.asarray(inputs[k], np.float32).T.copy() for k in ["W_diff", "W_same"]]
        + [np.asarray(inputs["res_W1"][i], np.float32).T.copy() for i in range(3)]
        + [np.asarray(inputs["res_W2"][i], np.float32).T.copy() for i in range(3)]
        + [np.asarray(inputs["W_last"], np.float32).T.copy()],
        axis=1).astype(np.float16)
    bpack = np.zeros((F, 16), np.float32)
    bpack[:, 1] = np.asarray(inputs["b_same"], np.float32)
    for i in range(3):
        bpack[:, 2 + i] = np.asarray(inputs["res_b1"][i], np.float32)
        bpack[:, 5 + i] = np.asarray(inputs["res_b2"][i], np.float32)
    bpack[:, 8] = np.asarray(inputs["b_last"], np.float32)
    bpack[:, 9] = np.asarray(inputs["u"], np.float32)[0]
    bdiv16 = np.tile((np.asarray(inputs["b_diff"], np.float32) / 128.0
                      ).astype(np.float16), (F, 1))
    G_w = np.asarray(inputs["G_w"], np.float32)
    gw2 = np.zeros((128, 128), np.float16)
    gw2[:K] = G_w.T.astype(np.float16)
    gw2[64:64 + K] = G_w.T.astype(np.float16)
    iota = np.broadcast_to(np.arange(CHUNK, dtype=np.float16), (128, CHUNK)).copy()
    xT16 = x.T.astype(np.float16).copy()
    xT32 = x.T.copy()

    in_maps = []
    for c in range(NC_):
        in_maps.append(dict(
            xT16=xT16, wpack16=wpack16, bpack=bpack, bdiv16=bdiv16, gw2=gw2,
            iota_in=iota, src_in=src_d[c], dstf_in=dstf_d[c], ea_in=ea_d[c],
            xTo=xT32[:, c * NSH:(c + 1) * NSH].copy(),
        ))
    res = bass_utils.run_bass_kernel_spmd(nc, in_maps, core_ids=list(range(NC_)))
    o0 = np.concatenate([res.results[c]["out0"] for c in range(NC_)], axis=0)
    o1 = np.concatenate([res.results[c]["out1"] for c in range(NC_)], axis=0)
    return (o0, o1)

